# revision 14
# baseline (speedup 1.0000x reference)
"""Trainium2 Bass kernel for nn_AttnMatching.

Reference computes:
    emb = emb_table[1:L+1]                      # [L, D]
    attn = einsum('ld,ntd->nlt', emb, self_attn)
    out  = einsum('nlt,t->nl', attn, value_w[0])

Reassociated (identical math):
    ctx[n, d] = sum_t value_w[t] * self_attn[n, t, d]    # [N, D]  (tiny)
    out[n, l] = sum_d ctx[n, d] * emb[l, d]              # [N, L]

Memory-bound: dominant traffic is streaming the embedding table.
Sharding: vocab axis L split across 8 cores (6250 cols each),
self_attn/value_w replicated, no communication.

All device traffic is bf16 (host-cast): emb 1.6 MB/core, attnw 0.44 MB,
out 0.2 MB. Matmuls run at bf16 rate (fp32 is 4 cycles/row + LOW_HIGH
double-pass). rel-err from bf16 ~3e-3, gate is 2e-2.

Per-core schedule (raw bacc, hand-rolled sems):
  - 3 DMA queues stream from the entry block: sync(HWDGE) carries the
    attnw burst (2 subs) then the last emb chunk; scalar(HWDGE) the
    first two emb chunks; gpsimd(SWDGE) the middle three.
  - ctx: per 8-n half, ONE fused multiply (scalar_tensor_tensor with a
    zero-stride broadcast AP repeating the w block) + ONE segmented
    tensor_reduce (axis=X over [128,8,100]) on DVE; ACT casts ctx_f32
    -> bf16 ctxT cross-engine (sem-gated: accumulator/pipeline drain).
  - PE: dependency-free bf16 warmups hold the clock ramp, then mains.
    Two schemes (K_SCHEME):
      wide: lhsT=ctxT [D,16] stationary, rhs=emb [D,512] -> PSUM
            [16,512] x13 over 6 banks; PSUM->SBUF copies round-robin
            DVE/ACT; out_sb repartitioned [64, 2048] (copy s writes
            partition base 16*(s%4)) so stores engage 8 SDMA engines.
      tp:   lhsT=emb tile [D,128] stationary, rhs=ctxT [D,16] moving ->
            PSUM [128,16] x49 packed into 2 banks; 2 full-width DVE
            copies; 2 full-rate [128,*] stores (host un-permutes).
  - Epilogue: sem-only all-engine barrier + semaphore clear so the NEFF
    is safe to re-execute.
"""

import os

import numpy as np

L = 50000
D = 128
T = 100
N = 16
NCORES = 8
LSH = L // NCORES          # 6250 columns per core

MM = 512                   # PSUM bank limit: fp32 out cols per matmul
SCHEME = os.environ.get("K_SCHEME", "wide")  # "wide" | "tp"
N_WARMUP = int(os.environ.get("K_N_WARMUP", "12"))
NUM_DEVICES = int(os.environ.get("K_NUM_DEVICES", str(NCORES)))
# wide-scheme out_sb layout: "shift" = [64, 2048] repartitioned (8-engine
# stores), "flat" = [16, LSH] (4-engine stores, no partition-shift copies)
OSB = os.environ.get("K_OSB", "shift")
# quad: matmul outputs land at 4 PSUM partition quadrants (tile_position via
# out AP base) so ONE 128-lane DVE copy moves 4 mm-units
QUAD = os.environ.get("K_QUAD", "1") == "1"

AW = T + N * T             # attnw cols: [w bcast | sa d-major]
NTILE = (LSH + 127) // 128          # 49 transposed tiles
LPAD = NTILE * 128                  # 6272: tp-scheme padded cols
NCOLS = {"wide": LSH, "tp": LPAD}

_cache = {}


def _chunks(total, step):
    return [(c0, min(c0 + step, total)) for c0 in range(0, total, step)]


def _view3(ap2d, ncols_inner, nrep, bcast=False):
    """[128, nrep*ncols_inner] slice -> [128, nrep, ncols_inner] view.
    bcast repeats the first ncols_inner cols nrep times (stride 0)."""
    from concourse.bass import AP

    pstep = ap2d.ap[0][0]
    step_rep = 0 if bcast else ncols_inner
    return AP(
        ap2d.tensor,
        ap2d.offset,
        [[pstep, 128], [step_rep, nrep], [1, ncols_inner]],
    )


def _build():
    import concourse.bacc as bacc
    import concourse.mybir as mybir

    f32 = mybir.dt.float32
    bf16 = mybir.dt.bfloat16

    nc = bacc.Bacc(
        "TRN2",
        target_bir_lowering=False,
        debug=False,
        enable_asserts=True,
        num_devices=NUM_DEVICES,
    )

    ncols = NCOLS[SCHEME]
    embT = nc.dram_tensor("embT", [D, ncols], bf16, kind="ExternalInput").ap()
    attnw = nc.dram_tensor("attnw", [D, AW], bf16, kind="ExternalInput").ap()
    if SCHEME == "wide":
        out_shape = ([96, 2560] if QUAD else [128, 2048]) if OSB == "shift" else [N, LSH]
    else:
        out_shape = [D, NTILE * N]
    out = nc.dram_tensor("out", out_shape, bf16, kind="ExternalOutput").ap()

    # emb chunks across the 3 DMA queues (1024-col granularity)
    bounds = [0, 1024, 2048, 3072, 4096, 5120, ncols]
    dma_chunks = list(zip(bounds[:-1], bounds[1:]))
    ring = {0: "scalar", 1: "scalar", 2: "gpsimd", 3: "gpsimd", 4: "sync", 5: "sync"}
    n_chunks = len(dma_chunks)

    # attnw sub-DMAs: sub0 = w + n0..7, sub1 = n8..15
    asub = [0, T + 8 * T, AW]

    # wide-scheme matmul list: (chunk_idx, abs_s0, abs_s1), 512-col units
    mm_list = []
    for ci, (c0, c1) in enumerate(dma_chunks):
        for s0, s1 in _chunks(c1 - c0, MM):
            mm_list.append((ci, c0 + s0, c0 + s1))
    NPS = 6
    CPENG = ["vector", "scalar"]  # copy engine per wide mm-unit (no PSUM on gpsimd)
    ne = len(CPENG)

    attnw_sb = nc.alloc_sbuf_tensor("attnw_sb", [D, AW], bf16).ap()
    emb_sb = [
        nc.alloc_sbuf_tensor(f"emb_sb{ci}", [D, c1 - c0], bf16).ap()
        for ci, (c0, c1) in enumerate(dma_chunks)
    ]
    wscr = nc.alloc_sbuf_tensor("wscr", [D, D + MM], bf16).ap()
    ctxT = nc.alloc_sbuf_tensor("ctxT", [D, N], bf16).ap()
    ctx_f32 = nc.alloc_sbuf_tensor("ctx_f32", [D, N], f32).ap()
    prod = nc.alloc_sbuf_tensor("prod", [D, N * T], bf16).ap()
    if SCHEME == "wide":
        out_sb = nc.alloc_sbuf_tensor("out_sb", out_shape, bf16).ap()
    else:
        out_sb = nc.alloc_sbuf_tensor("out_sb", [D, NTILE * N], bf16).ap()

    ps_warm = nc.alloc_psum_tensor("ps_warm", [D, MM], f32).ap()
    if SCHEME == "wide":
        if QUAD:
            ps_main = [
                nc.alloc_psum_tensor(f"ps_q{j}", [D, MM], f32).ap() for j in range(3)
            ]
        else:
            ps_main = [
                nc.alloc_psum_tensor(f"ps_main{j}", [N, MM], f32).ap()
                for j in range(NPS)
            ]
    else:
        ps_tp = [
            nc.alloc_psum_tensor("ps_tp0", [D, MM], f32).ap(),
            nc.alloc_psum_tensor("ps_tp1", [D, (NTILE - 32) * N], f32).ap(),
        ]

    lda = [nc.alloc_semaphore(f"lda{g}") for g in range(2)]
    lde = [nc.alloc_semaphore(f"lde{ci}") for ci in range(n_chunks)]
    z = nc.alloc_semaphore("z")
    cxr = nc.alloc_semaphore("cxr")
    cxv = nc.alloc_semaphore("cxv")
    mm_sem = nc.alloc_semaphore("mm")
    cp = {k: nc.alloc_semaphore(f"cp_{k}") for k in CPENG}
    st = nc.alloc_semaphore("st")
    all_sems = lda + lde + [z, cxr, cxv, mm_sem] + list(cp.values()) + [st]

    # ---- entry block: all unconditional DMA issues + warmup memset ----
    for g in range(2):
        nc.sync.dma_start(
            attnw_sb[:, asub[g] : asub[g + 1]], attnw[:, asub[g] : asub[g + 1]]
        ).then_inc(lda[g], 16)
    for ci, (c0, c1) in enumerate(dma_chunks):
        eng = getattr(nc, ring[ci])
        d = eng.dma_start(emb_sb[ci][:, :], embT[:, c0:c1]).then_inc(lde[ci], 16)
        d._wait_ge(lda[1], 16)  # attnw gets a solo SDMA window first
    nc.vector.memset(wscr[:, :], 0.0).then_inc(z, 1)

    def ctx_stt(h):
        """prod half h = attnw[n-half h] * w (one STT with w broadcast)."""
        in0 = _view3(attnw_sb[:, T + h * 8 * T : T + (h + 1) * 8 * T], T, 8)
        w_b = _view3(attnw_sb[:, 0:T], T, 8, bcast=True)
        pv = _view3(prod[:, h * 8 * T :], T, 8)
        return nc.vector.scalar_tensor_tensor(
            pv, in0, 1.0, w_b,
            op0=mybir.AluOpType.bypass,
            op1=mybir.AluOpType.mult,
        )

    def ctx_reduce():
        return nc.vector.tensor_reduce(
            ctx_f32[:, :], _view3(prod[:, :], T, N),
            axis=mybir.AxisListType.X, op=mybir.AluOpType.add,
        )

    # wide+shift: copy for mm-unit s lands at partition base 32*(s%4) (engine
    # writes must start on a quadrant boundary), col block 512*(s//4); stores
    # then read all partition quadrants (16 SDMA engines, half-garbage rows)
    def osb_dst(s, width):
        if OSB == "shift":
            a, b = s % 4, s // 4
            return out_sb[32 * a : 32 * a + 16, MM * b : MM * b + width]
        ci, s0, s1 = mm_list[s]
        return out_sb[:, s0 : s0 + width]

    with nc.Block() as block:

        @block.vector
        def _(v):
            v.wait_ge(lda[0], 16)
            ctx_stt(0)
            v.wait_ge(lda[1], 16)
            ctx_stt(1)
            ctx_reduce().then_inc(cxr, 1)
            if SCHEME == "wide":
                if QUAD:
                    for g in range(5):
                        hi = min(3 * (g + 1), len(mm_list))
                        v.wait_ge(mm_sem, hi)
                        nc.vector.tensor_copy(
                            out_sb[:, MM * g : MM * (g + 1)], ps_main[g % 3][:96, :]
                        ).then_inc(cp["vector"], 1)
                else:
                    for s, (ci, s0, s1) in enumerate(mm_list):
                        if CPENG[s % ne] != "vector":
                            continue
                        v.wait_ge(mm_sem, s + 1)
                        nc.vector.tensor_copy(
                            osb_dst(s, s1 - s0), ps_main[s % NPS][:, : s1 - s0]
                        ).then_inc(cp["vector"], 1)
            else:
                v.wait_ge(mm_sem, 32)
                nc.vector.tensor_copy(out_sb[:, : 32 * N], ps_tp[0][:, :]).then_inc(
                    cp["vector"], 1
                )
                v.wait_ge(mm_sem, NTILE)
                nc.vector.tensor_copy(out_sb[:, 32 * N :], ps_tp[1][:, :]).then_inc(
                    cp["vector"], 1
                )

        @block.tensor
        def _(t):
            t.wait_ge(z, 1)
            for _wi in range(N_WARMUP):
                nc.tensor.matmul(
                    ps_warm[:, :],
                    lhsT=wscr[:, :D],
                    rhs=wscr[:, D:],
                    start=True,
                    stop=True,
                )
            t.wait_ge(cxv, 1)
            if SCHEME == "wide":
                prev_ci = -1
                for s, (ci, s0, s1) in enumerate(mm_list):
                    if ci != prev_ci:
                        t.wait_ge(lde[ci], 16)
                        prev_ci = ci
                    if QUAD:
                        g, a = s // 3, s % 3
                        if a == 0 and g >= 3:
                            t.wait_ge(cp["vector"], g - 2)
                        dst = ps_main[g % 3][32 * a : 32 * a + 16, : s1 - s0]
                    else:
                        if s >= NPS:
                            q = s - NPS
                            t.wait_ge(cp[CPENG[q % ne]], q // ne + 1)
                        dst = ps_main[s % NPS][:, : s1 - s0]
                    c0 = dma_chunks[ci][0]
                    nc.tensor.matmul(
                        dst,
                        lhsT=ctxT[:, :],
                        rhs=emb_sb[ci][:, s0 - c0 : s1 - c0],
                        start=True,
                        stop=True,
                    ).then_inc(mm_sem, 1)
            else:
                prev_ci = -1
                for ti in range(NTILE):
                    ci = min(ti // 8, n_chunks - 1)
                    if ci != prev_ci:
                        t.wait_ge(lde[ci], 16)
                        prev_ci = ci
                    c0 = dma_chunks[ci][0]
                    bank, off = (0, ti) if ti < 32 else (1, ti - 32)
                    nc.tensor.matmul(
                        ps_tp[bank][:, off * N : (off + 1) * N],
                        lhsT=emb_sb[ci][:, ti * 128 - c0 : ti * 128 - c0 + 128],
                        rhs=ctxT[:, :],
                        start=True,
                        stop=True,
                    ).then_inc(mm_sem, 1)

        @block.scalar
        def _(sc):
            # ctx cast runs cross-engine: guarantees DVE reduce has drained
            sc.wait_ge(cxr, 1)
            nc.scalar.copy(ctxT[:, :], ctx_f32[:, :]).then_inc(cxv, 1)
            if SCHEME == "wide" and QUAD:
                for g in range(5):
                    sc.wait_ge(cp["vector"], g + 1)
                    nc.scalar.dma_start(
                        out[:, MM * g : MM * (g + 1)],
                        out_sb[:, MM * g : MM * (g + 1)],
                    ).then_inc(st, 16)
            elif SCHEME == "wide":
                acts = [s for s in range(len(mm_list)) if CPENG[s % ne] == "scalar"]
                if OSB == "shift":
                    # store1: mm-units 0-7 (col blocks 0-1); store2: 8-12
                    store_plan = [(0, 8, 0, 1024), (8, 13, 1024, 2048)]
                else:
                    store_plan = [(0, 3, 0, 1536), (3, 6, 1536, 3072),
                                  (6, 9, 3072, 4608), (9, 13, 4608, LSH)]
                done = {k: 0 for k in CPENG}
                ai = 0
                for q0, q1, b0, b1 in store_plan:
                    while ai < len(acts) and acts[ai] < q1:
                        s = acts[ai]
                        ci, s0, s1 = mm_list[s]
                        sc.wait_ge(mm_sem, s + 1)
                        nc.scalar.copy(
                            osb_dst(s, s1 - s0), ps_main[s % NPS][:, : s1 - s0]
                        ).then_inc(cp["scalar"], 1)
                        done["scalar"] += 1
                        ai += 1
                    for s in range(q0, q1):
                        k = CPENG[s % ne]
                        need = s // ne + 1
                        if k != "scalar" and need > done[k]:
                            sc.wait_ge(cp[k], need)
                            done[k] = need
                    if OSB == "shift":
                        nc.scalar.dma_start(
                            out[:, b0:b1], out_sb[:, b0:b1]
                        ).then_inc(st, 16)
                    else:
                        nc.scalar.dma_start(
                            out[:, b0:b1], out_sb[:, b0:b1]
                        ).then_inc(st, 16)
            else:
                sc.wait_ge(cp["vector"], 1)
                nc.scalar.dma_start(out[:, : 32 * N], out_sb[:, : 32 * N]).then_inc(
                    st, 16
                )
                sc.wait_ge(cp["vector"], 2)
                nc.scalar.dma_start(out[:, 32 * N :], out_sb[:, 32 * N :]).then_inc(
                    st, 16
                )
            # no completion wait: epilogue dma_reset drains the store queue

    nc.all_engine_barrier(sem_only=True)
    nc.clear_and_free_semaphores(all_sems)

    nc.compile()
    return nc


def _get_nc():
    if "nc" not in _cache:
        _cache["nc"] = _build()
    return _cache["nc"]


def _make_in_maps(self_attn, emb_table, value_w):
    import ml_dtypes

    bf = ml_dtypes.bfloat16
    self_attn = np.asarray(self_attn, dtype=np.float32)
    value_w = np.asarray(value_w, dtype=np.float32)
    # [D, T + N*T]: value_w broadcast, then d-major self_attn
    attnw = np.empty((D, AW), dtype=bf)
    attnw[:, :T] = value_w[0][None, :].astype(bf)
    attnw[:, T:] = self_attn.transpose(2, 0, 1).reshape(D, N * T).astype(bf)
    embT = np.asarray(emb_table, dtype=np.float32)[1 : L + 1].astype(bf).T  # [D, L]
    ncols = NCOLS[SCHEME]
    in_maps = []
    for k in range(NCORES):
        shard = np.zeros((D, ncols), dtype=bf)
        shard[:, :LSH] = embT[:, k * LSH : (k + 1) * LSH]
        in_maps.append({"embT": shard, "attnw": attnw})
    return in_maps


def _unshard(o):
    o = np.asarray(o)
    if SCHEME == "wide":
        if OSB == "shift":
            # quad: dram[32a+n, 512g+j] = out[n, 512*(3g+a)+j]; legacy shift
            # layout uses 4-unit groups on [128, 2048]
            na = 3 if QUAD else 4
            full = np.empty((N, LSH), dtype=np.float32)
            for s in range(13):
                a, b = s % na, s // na
                w = min(MM, LSH - s * MM)
                full[:, s * MM : s * MM + w] = o[
                    32 * a : 32 * a + 16, MM * b : MM * b + w
                ].astype(np.float32)
            return full
        return o.astype(np.float32)
    # tp: [128, 49*16] -> [49,128,16] l-major -> [LSH, N] -> [N, LSH]
    return (
        o.reshape(D, NTILE, N)
        .transpose(1, 0, 2)
        .reshape(LPAD, N)[:LSH]
        .T.astype(np.float32)
    )


def run(self_attn, emb_table, value_w, trace=False):
    from concourse.bass_utils import run_bass_kernel_spmd

    nc = _get_nc()
    in_maps = _make_in_maps(self_attn, emb_table, value_w)
    res = run_bass_kernel_spmd(nc, in_maps, list(range(NCORES)), trace=trace)
    full = np.ascontiguousarray(
        np.concatenate([_unshard(res.results[k]["out"]) for k in range(NCORES)], axis=1),
        dtype=np.float32,
    )
    return full, res


def kernel(self_attn, mat2, traj, emb_table, value_w):
    full, _ = run(self_attn, emb_table, value_w, trace=False)
    return full


# revision 15
# speedup vs baseline: 1.2341x; 1.2341x over previous
"""Trainium2 Bass kernel for nn_AttnMatching.

Reference computes:
    emb = emb_table[1:L+1]                      # [L, D]
    attn = einsum('ld,ntd->nlt', emb, self_attn)
    out  = einsum('nlt,t->nl', attn, value_w[0])

Reassociated (identical math):
    ctx[n, d] = sum_t value_w[t] * self_attn[n, t, d]    # [N, D]  (tiny)
    out[n, l] = sum_d ctx[n, d] * emb[l, d]              # [N, L]

Memory-bound: dominant traffic is streaming the embedding table.
Sharding: vocab axis L split across 8 cores (6250 cols each),
self_attn/value_w replicated, no communication.

All device traffic is bf16 (host-cast): emb 1.6 MB/core, attnw 0.44 MB,
out 0.2 MB. Matmuls run at bf16 rate (fp32 is 4 cycles/row + LOW_HIGH
double-pass). rel-err from bf16 ~3e-3, gate is 2e-2.

Per-core schedule (raw bacc, hand-rolled sems):
  - 3 DMA queues stream from the entry block: sync(HWDGE) carries the
    attnw burst (2 subs) then the last emb chunk; scalar(HWDGE) the
    first two emb chunks; gpsimd(SWDGE) the middle three.
  - ctx: per 8-n half, ONE fused multiply (scalar_tensor_tensor with a
    zero-stride broadcast AP repeating the w block) + ONE segmented
    tensor_reduce (axis=X over [128,8,100]) on DVE; ACT casts ctx_f32
    -> bf16 ctxT cross-engine (sem-gated: accumulator/pipeline drain).
  - PE: dependency-free bf16 warmups hold the clock ramp, then mains.
    Two schemes (K_SCHEME):
      wide: lhsT=ctxT [D,16] stationary, rhs=emb [D,512] -> PSUM
            [16,512] x13 over 6 banks; PSUM->SBUF copies round-robin
            DVE/ACT; out_sb repartitioned [64, 2048] (copy s writes
            partition base 16*(s%4)) so stores engage 8 SDMA engines.
      tp:   lhsT=emb tile [D,128] stationary, rhs=ctxT [D,16] moving ->
            PSUM [128,16] x49 packed into 2 banks; 2 full-width DVE
            copies; 2 full-rate [128,*] stores (host un-permutes).
  - Epilogue: sem-only all-engine barrier + semaphore clear so the NEFF
    is safe to re-execute.
"""

import os

import numpy as np

L = 50000
D = 128
T = 100
N = 16
NCORES = 8
LSH = L // NCORES          # 6250 columns per core

MM = 512                   # PSUM bank limit: fp32 out cols per matmul
SCHEME = os.environ.get("K_SCHEME", "wide")  # "wide" | "tp"
N_WARMUP = int(os.environ.get("K_N_WARMUP", "15"))
NUM_DEVICES = int(os.environ.get("K_NUM_DEVICES", str(NCORES)))
# wide-scheme out_sb layout: "shift" = [64, 2048] repartitioned (8-engine
# stores), "flat" = [16, LSH] (4-engine stores, no partition-shift copies)
OSB = os.environ.get("K_OSB", "shift")
# quad: matmul outputs land at 4 PSUM partition quadrants (tile_position via
# out AP base) so ONE 128-lane DVE copy moves 4 mm-units
QUAD = os.environ.get("K_QUAD", "1") == "1"

AW = T + N * T             # attnw cols: [w bcast | sa d-major]
NTILE = (LSH + 127) // 128          # 49 transposed tiles
LPAD = NTILE * 128                  # 6272: tp-scheme padded cols
NCOLS = {"wide": LSH, "tp": LPAD}

_cache = {}


def _chunks(total, step):
    return [(c0, min(c0 + step, total)) for c0 in range(0, total, step)]


def _view3(ap2d, ncols_inner, nrep, bcast=False):
    """[128, nrep*ncols_inner] slice -> [128, nrep, ncols_inner] view.
    bcast repeats the first ncols_inner cols nrep times (stride 0)."""
    from concourse.bass import AP

    pstep = ap2d.ap[0][0]
    step_rep = 0 if bcast else ncols_inner
    return AP(
        ap2d.tensor,
        ap2d.offset,
        [[pstep, 128], [step_rep, nrep], [1, ncols_inner]],
    )


def _build():
    import concourse.bacc as bacc
    import concourse.mybir as mybir

    f32 = mybir.dt.float32
    bf16 = mybir.dt.bfloat16

    nc = bacc.Bacc(
        "TRN2",
        target_bir_lowering=False,
        debug=False,
        enable_asserts=True,
        num_devices=NUM_DEVICES,
    )

    ncols = NCOLS[SCHEME]
    embT = nc.dram_tensor("embT", [D, ncols], bf16, kind="ExternalInput").ap()
    attnw = nc.dram_tensor("attnw", [D, AW], bf16, kind="ExternalInput").ap()
    if SCHEME == "wide":
        out_shape = ([96, 2560] if QUAD else [128, 2048]) if OSB == "shift" else [N, LSH]
    else:
        out_shape = [D, NTILE * N]
    out = nc.dram_tensor("out", out_shape, bf16, kind="ExternalOutput").ap()

    # emb chunks across the 3 DMA queues (1024-col granularity)
    bounds = [0, 1024, 2048, 3072, 4096, 5120, ncols]
    dma_chunks = list(zip(bounds[:-1], bounds[1:]))
    ring = {0: "scalar", 1: "scalar", 2: "gpsimd", 3: "gpsimd", 4: "sync", 5: "sync"}
    n_chunks = len(dma_chunks)

    # attnw sub-DMAs: sub0 = w + n0..7, sub1 = n8..15
    asub = [0, T + 8 * T, AW]

    # wide-scheme matmul list: (chunk_idx, abs_s0, abs_s1), 512-col units
    mm_list = []
    for ci, (c0, c1) in enumerate(dma_chunks):
        for s0, s1 in _chunks(c1 - c0, MM):
            mm_list.append((ci, c0 + s0, c0 + s1))
    NPS = 6
    CPENG = ["vector", "scalar"]  # copy engine per wide mm-unit (no PSUM on gpsimd)
    ne = len(CPENG)

    attnw_sb = nc.alloc_sbuf_tensor("attnw_sb", [D, AW], bf16).ap()
    emb_sb = [
        nc.alloc_sbuf_tensor(f"emb_sb{ci}", [D, c1 - c0], bf16).ap()
        for ci, (c0, c1) in enumerate(dma_chunks)
    ]
    wscr = nc.alloc_sbuf_tensor("wscr", [D, D + MM], bf16).ap()
    ctxT = nc.alloc_sbuf_tensor("ctxT", [D, N], bf16).ap()
    ctx_f32 = nc.alloc_sbuf_tensor("ctx_f32", [D, N], f32).ap()
    prod = nc.alloc_sbuf_tensor("prod", [D, N * T], bf16).ap()
    if SCHEME == "wide":
        out_sb = nc.alloc_sbuf_tensor("out_sb", out_shape, bf16).ap()
    else:
        out_sb = nc.alloc_sbuf_tensor("out_sb", [D, NTILE * N], bf16).ap()

    ps_warm = nc.alloc_psum_tensor("ps_warm", [D, MM], f32).ap()
    if SCHEME == "wide":
        if QUAD:
            ps_main = [
                nc.alloc_psum_tensor(f"ps_q{j}", [D, MM], f32).ap() for j in range(3)
            ]
        else:
            ps_main = [
                nc.alloc_psum_tensor(f"ps_main{j}", [N, MM], f32).ap()
                for j in range(NPS)
            ]
    else:
        ps_tp = [
            nc.alloc_psum_tensor("ps_tp0", [D, MM], f32).ap(),
            nc.alloc_psum_tensor("ps_tp1", [D, (NTILE - 32) * N], f32).ap(),
        ]

    lda = [nc.alloc_semaphore(f"lda{g}") for g in range(2)]
    lde = [nc.alloc_semaphore(f"lde{ci}") for ci in range(n_chunks)]
    z = nc.alloc_semaphore("z")
    cxr = nc.alloc_semaphore("cxr")
    cxv = nc.alloc_semaphore("cxv")
    mm_sem = nc.alloc_semaphore("mm")
    cp = {k: nc.alloc_semaphore(f"cp_{k}") for k in CPENG}
    st = nc.alloc_semaphore("st")
    all_sems = lda + lde + [z, cxr, cxv, mm_sem] + list(cp.values()) + [st]

    # ---- entry block: all unconditional DMA issues + warmup memset ----
    for g, eng in enumerate((nc.sync, nc.scalar)):
        eng.dma_start(
            attnw_sb[:, asub[g] : asub[g + 1]], attnw[:, asub[g] : asub[g + 1]]
        ).then_inc(lda[g], 16)
    for ci, (c0, c1) in enumerate(dma_chunks):
        eng = getattr(nc, ring[ci])
        eng.dma_start(emb_sb[ci][:, :], embT[:, c0:c1]).then_inc(lde[ci], 16)
    nc.vector.memset(wscr[:, :], 0.0).then_inc(z, 1)

    def ctx_stt(h):
        """prod half h = attnw[n-half h] * w (one STT with w broadcast)."""
        in0 = _view3(attnw_sb[:, T + h * 8 * T : T + (h + 1) * 8 * T], T, 8)
        w_b = _view3(attnw_sb[:, 0:T], T, 8, bcast=True)
        pv = _view3(prod[:, h * 8 * T :], T, 8)
        return nc.vector.scalar_tensor_tensor(
            pv, in0, 1.0, w_b,
            op0=mybir.AluOpType.bypass,
            op1=mybir.AluOpType.mult,
        )

    def ctx_reduce():
        return nc.vector.tensor_reduce(
            ctx_f32[:, :], _view3(prod[:, :], T, N),
            axis=mybir.AxisListType.X, op=mybir.AluOpType.add,
        )

    # wide+shift: copy for mm-unit s lands at partition base 32*(s%4) (engine
    # writes must start on a quadrant boundary), col block 512*(s//4); stores
    # then read all partition quadrants (16 SDMA engines, half-garbage rows)
    def osb_dst(s, width):
        if OSB == "shift":
            a, b = s % 4, s // 4
            return out_sb[32 * a : 32 * a + 16, MM * b : MM * b + width]
        ci, s0, s1 = mm_list[s]
        return out_sb[:, s0 : s0 + width]

    with nc.Block() as block:

        @block.vector
        def _(v):
            v.wait_ge(lda[0], 16)
            ctx_stt(0)
            v.wait_ge(lda[1], 16)
            ctx_stt(1)
            ctx_reduce().then_inc(cxr, 1)
            if SCHEME == "wide":
                if QUAD:
                    for g in range(5):
                        hi = min(3 * (g + 1), len(mm_list))
                        v.wait_ge(mm_sem, hi)
                        nc.vector.tensor_copy(
                            out_sb[:, MM * g : MM * (g + 1)], ps_main[g % 3][:96, :]
                        ).then_inc(cp["vector"], 1)
                else:
                    for s, (ci, s0, s1) in enumerate(mm_list):
                        if CPENG[s % ne] != "vector":
                            continue
                        v.wait_ge(mm_sem, s + 1)
                        nc.vector.tensor_copy(
                            osb_dst(s, s1 - s0), ps_main[s % NPS][:, : s1 - s0]
                        ).then_inc(cp["vector"], 1)
            else:
                v.wait_ge(mm_sem, 32)
                nc.vector.tensor_copy(out_sb[:, : 32 * N], ps_tp[0][:, :]).then_inc(
                    cp["vector"], 1
                )
                v.wait_ge(mm_sem, NTILE)
                nc.vector.tensor_copy(out_sb[:, 32 * N :], ps_tp[1][:, :]).then_inc(
                    cp["vector"], 1
                )

        @block.tensor
        def _(t):
            t.wait_ge(z, 1)
            for _wi in range(N_WARMUP):
                nc.tensor.matmul(
                    ps_warm[:, :],
                    lhsT=wscr[:, :D],
                    rhs=wscr[:, D:],
                    start=True,
                    stop=True,
                )
            t.wait_ge(cxv, 1)
            if SCHEME == "wide":
                prev_ci = -1
                for s, (ci, s0, s1) in enumerate(mm_list):
                    if ci != prev_ci:
                        t.wait_ge(lde[ci], 16)
                        prev_ci = ci
                    if QUAD:
                        g, a = s // 3, s % 3
                        if a == 0 and g >= 3:
                            t.wait_ge(cp["vector"], g - 2)
                        dst = ps_main[g % 3][32 * a : 32 * a + 16, : s1 - s0]
                    else:
                        if s >= NPS:
                            q = s - NPS
                            t.wait_ge(cp[CPENG[q % ne]], q // ne + 1)
                        dst = ps_main[s % NPS][:, : s1 - s0]
                    c0 = dma_chunks[ci][0]
                    nc.tensor.matmul(
                        dst,
                        lhsT=ctxT[:, :],
                        rhs=emb_sb[ci][:, s0 - c0 : s1 - c0],
                        start=True,
                        stop=True,
                    ).then_inc(mm_sem, 1)
            else:
                prev_ci = -1
                for ti in range(NTILE):
                    ci = min(ti // 8, n_chunks - 1)
                    if ci != prev_ci:
                        t.wait_ge(lde[ci], 16)
                        prev_ci = ci
                    c0 = dma_chunks[ci][0]
                    bank, off = (0, ti) if ti < 32 else (1, ti - 32)
                    nc.tensor.matmul(
                        ps_tp[bank][:, off * N : (off + 1) * N],
                        lhsT=emb_sb[ci][:, ti * 128 - c0 : ti * 128 - c0 + 128],
                        rhs=ctxT[:, :],
                        start=True,
                        stop=True,
                    ).then_inc(mm_sem, 1)

        @block.scalar
        def _(sc):
            # ctx cast runs cross-engine: guarantees DVE reduce has drained
            sc.wait_ge(cxr, 1)
            nc.scalar.copy(ctxT[:, :], ctx_f32[:, :]).then_inc(cxv, 1)
            if SCHEME == "wide" and QUAD:
                for g in range(5):
                    sc.wait_ge(cp["vector"], g + 1)
                    nc.scalar.dma_start(
                        out[:, MM * g : MM * (g + 1)],
                        out_sb[:, MM * g : MM * (g + 1)],
                    ).then_inc(st, 16)
            elif SCHEME == "wide":
                acts = [s for s in range(len(mm_list)) if CPENG[s % ne] == "scalar"]
                if OSB == "shift":
                    # store1: mm-units 0-7 (col blocks 0-1); store2: 8-12
                    store_plan = [(0, 8, 0, 1024), (8, 13, 1024, 2048)]
                else:
                    store_plan = [(0, 3, 0, 1536), (3, 6, 1536, 3072),
                                  (6, 9, 3072, 4608), (9, 13, 4608, LSH)]
                done = {k: 0 for k in CPENG}
                ai = 0
                for q0, q1, b0, b1 in store_plan:
                    while ai < len(acts) and acts[ai] < q1:
                        s = acts[ai]
                        ci, s0, s1 = mm_list[s]
                        sc.wait_ge(mm_sem, s + 1)
                        nc.scalar.copy(
                            osb_dst(s, s1 - s0), ps_main[s % NPS][:, : s1 - s0]
                        ).then_inc(cp["scalar"], 1)
                        done["scalar"] += 1
                        ai += 1
                    for s in range(q0, q1):
                        k = CPENG[s % ne]
                        need = s // ne + 1
                        if k != "scalar" and need > done[k]:
                            sc.wait_ge(cp[k], need)
                            done[k] = need
                    if OSB == "shift":
                        nc.scalar.dma_start(
                            out[:, b0:b1], out_sb[:, b0:b1]
                        ).then_inc(st, 16)
                    else:
                        nc.scalar.dma_start(
                            out[:, b0:b1], out_sb[:, b0:b1]
                        ).then_inc(st, 16)
            else:
                sc.wait_ge(cp["vector"], 1)
                nc.scalar.dma_start(out[:, : 32 * N], out_sb[:, : 32 * N]).then_inc(
                    st, 16
                )
                sc.wait_ge(cp["vector"], 2)
                nc.scalar.dma_start(out[:, 32 * N :], out_sb[:, 32 * N :]).then_inc(
                    st, 16
                )
            # no completion wait: epilogue dma_reset drains the store queue

    nc.all_engine_barrier(sem_only=True)
    nc.clear_and_free_semaphores(all_sems)

    nc.compile()
    return nc


def _get_nc():
    if "nc" not in _cache:
        _cache["nc"] = _build()
    return _cache["nc"]


def _make_in_maps(self_attn, emb_table, value_w):
    import ml_dtypes

    bf = ml_dtypes.bfloat16
    self_attn = np.asarray(self_attn, dtype=np.float32)
    value_w = np.asarray(value_w, dtype=np.float32)
    # [D, T + N*T]: value_w broadcast, then d-major self_attn
    attnw = np.empty((D, AW), dtype=bf)
    attnw[:, :T] = value_w[0][None, :].astype(bf)
    attnw[:, T:] = self_attn.transpose(2, 0, 1).reshape(D, N * T).astype(bf)
    embT = np.asarray(emb_table, dtype=np.float32)[1 : L + 1].astype(bf).T  # [D, L]
    ncols = NCOLS[SCHEME]
    in_maps = []
    for k in range(NCORES):
        shard = np.zeros((D, ncols), dtype=bf)
        shard[:, :LSH] = embT[:, k * LSH : (k + 1) * LSH]
        in_maps.append({"embT": shard, "attnw": attnw})
    return in_maps


def _unshard(o):
    o = np.asarray(o)
    if SCHEME == "wide":
        if OSB == "shift":
            # quad: dram[32a+n, 512g+j] = out[n, 512*(3g+a)+j]; legacy shift
            # layout uses 4-unit groups on [128, 2048]
            na = 3 if QUAD else 4
            full = np.empty((N, LSH), dtype=np.float32)
            for s in range(13):
                a, b = s % na, s // na
                w = min(MM, LSH - s * MM)
                full[:, s * MM : s * MM + w] = o[
                    32 * a : 32 * a + 16, MM * b : MM * b + w
                ].astype(np.float32)
            return full
        return o.astype(np.float32)
    # tp: [128, 49*16] -> [49,128,16] l-major -> [LSH, N] -> [N, LSH]
    return (
        o.reshape(D, NTILE, N)
        .transpose(1, 0, 2)
        .reshape(LPAD, N)[:LSH]
        .T.astype(np.float32)
    )


def run(self_attn, emb_table, value_w, trace=False):
    from concourse.bass_utils import run_bass_kernel_spmd

    nc = _get_nc()
    in_maps = _make_in_maps(self_attn, emb_table, value_w)
    res = run_bass_kernel_spmd(nc, in_maps, list(range(NCORES)), trace=trace)
    full = np.ascontiguousarray(
        np.concatenate([_unshard(res.results[k]["out"]) for k in range(NCORES)], axis=1),
        dtype=np.float32,
    )
    return full, res


def kernel(self_attn, mat2, traj, emb_table, value_w):
    full, _ = run(self_attn, emb_table, value_w, trace=False)
    return full


# revision 16
# speedup vs baseline: 1.2446x; 1.0085x over previous
"""Trainium2 Bass kernel for nn_AttnMatching.

Reference computes:
    emb = emb_table[1:L+1]                      # [L, D]
    attn = einsum('ld,ntd->nlt', emb, self_attn)
    out  = einsum('nlt,t->nl', attn, value_w[0])

Reassociated (identical math):
    ctx[n, d] = sum_t value_w[t] * self_attn[n, t, d]    # [N, D]  (tiny)
    out[n, l] = sum_d ctx[n, d] * emb[l, d]              # [N, L]

Memory-bound: dominant traffic is streaming the embedding table.
Sharding: vocab axis L split across 8 cores (6250 cols each),
self_attn/value_w replicated, no communication.

All device traffic is bf16 (host-cast): emb 1.6 MB/core, attnw 0.44 MB,
out 0.2 MB. Matmuls run at bf16 rate (fp32 is 4 cycles/row + LOW_HIGH
double-pass). rel-err from bf16 ~3e-3, gate is 2e-2.

Per-core schedule (raw bacc, hand-rolled sems):
  - 3 DMA queues stream from the entry block: sync(HWDGE) carries the
    attnw burst (2 subs) then the last emb chunk; scalar(HWDGE) the
    first two emb chunks; gpsimd(SWDGE) the middle three.
  - ctx: per 8-n half, ONE fused multiply (scalar_tensor_tensor with a
    zero-stride broadcast AP repeating the w block) + ONE segmented
    tensor_reduce (axis=X over [128,8,100]) on DVE; ACT casts ctx_f32
    -> bf16 ctxT cross-engine (sem-gated: accumulator/pipeline drain).
  - PE: dependency-free bf16 warmups hold the clock ramp, then mains.
    Two schemes (K_SCHEME):
      wide: lhsT=ctxT [D,16] stationary, rhs=emb [D,512] -> PSUM
            [16,512] x13 over 6 banks; PSUM->SBUF copies round-robin
            DVE/ACT; out_sb repartitioned [64, 2048] (copy s writes
            partition base 16*(s%4)) so stores engage 8 SDMA engines.
      tp:   lhsT=emb tile [D,128] stationary, rhs=ctxT [D,16] moving ->
            PSUM [128,16] x49 packed into 2 banks; 2 full-width DVE
            copies; 2 full-rate [128,*] stores (host un-permutes).
  - Epilogue: sem-only all-engine barrier + semaphore clear so the NEFF
    is safe to re-execute.
"""

import os

import numpy as np

L = 50000
D = 128
T = 100
N = 16
NCORES = 8
LSH = L // NCORES          # 6250 columns per core

MM = 512                   # PSUM bank limit: fp32 out cols per matmul
SCHEME = os.environ.get("K_SCHEME", "wide")  # "wide" | "tp"
N_WARMUP = int(os.environ.get("K_N_WARMUP", "12"))
NUM_DEVICES = int(os.environ.get("K_NUM_DEVICES", str(NCORES)))
# wide-scheme out_sb layout: "shift" = [64, 2048] repartitioned (8-engine
# stores), "flat" = [16, LSH] (4-engine stores, no partition-shift copies)
OSB = os.environ.get("K_OSB", "shift")
# quad: matmul outputs land at 4 PSUM partition quadrants (tile_position via
# out AP base) so ONE 128-lane DVE copy moves 4 mm-units
QUAD = os.environ.get("K_QUAD", "1") == "1"

AW = T + N * T             # attnw cols: [w bcast | sa d-major]
NTILE = (LSH + 127) // 128          # 49 transposed tiles
LPAD = NTILE * 128                  # 6272: tp-scheme padded cols
NCOLS = {"wide": LSH, "tp": LPAD}

_cache = {}


def _chunks(total, step):
    return [(c0, min(c0 + step, total)) for c0 in range(0, total, step)]


def _view3(ap2d, ncols_inner, nrep, bcast=False):
    """[128, nrep*ncols_inner] slice -> [128, nrep, ncols_inner] view.
    bcast repeats the first ncols_inner cols nrep times (stride 0)."""
    from concourse.bass import AP

    pstep = ap2d.ap[0][0]
    step_rep = 0 if bcast else ncols_inner
    return AP(
        ap2d.tensor,
        ap2d.offset,
        [[pstep, 128], [step_rep, nrep], [1, ncols_inner]],
    )


def _build():
    import concourse.bacc as bacc
    import concourse.mybir as mybir

    f32 = mybir.dt.float32
    bf16 = mybir.dt.bfloat16

    nc = bacc.Bacc(
        "TRN2",
        target_bir_lowering=False,
        debug=False,
        enable_asserts=True,
        num_devices=NUM_DEVICES,
    )

    ncols = NCOLS[SCHEME]
    embT = nc.dram_tensor("embT", [D, ncols], bf16, kind="ExternalInput").ap()
    attnw = nc.dram_tensor("attnw", [D, AW], bf16, kind="ExternalInput").ap()
    if SCHEME == "wide":
        out_shape = ([80, 2560] if QUAD else [128, 2048]) if OSB == "shift" else [N, LSH]
    else:
        out_shape = [D, NTILE * N]
    out = nc.dram_tensor("out", out_shape, bf16, kind="ExternalOutput").ap()

    # emb chunks across the 3 DMA queues (1024-col granularity)
    bounds = [0, 1024, 2048, 3072, 4096, 5120, ncols]
    dma_chunks = list(zip(bounds[:-1], bounds[1:]))
    ring = {0: "sync", 1: "sync", 2: "gpsimd", 3: "gpsimd", 4: "gpsimd", 5: "scalar"}
    n_chunks = len(dma_chunks)

    # attnw sub-DMAs: sub0 = w + n0..7, sub1 = n8..15
    asub = [0, T + 8 * T, AW]

    # wide-scheme matmul list: (chunk_idx, abs_s0, abs_s1), 512-col units
    mm_list = []
    for ci, (c0, c1) in enumerate(dma_chunks):
        for s0, s1 in _chunks(c1 - c0, MM):
            mm_list.append((ci, c0 + s0, c0 + s1))
    NPS = 6
    CPENG = ["vector", "scalar"]  # copy engine per wide mm-unit (no PSUM on gpsimd)
    ne = len(CPENG)

    attnw_sb = nc.alloc_sbuf_tensor("attnw_sb", [D, AW], bf16).ap()
    emb_sb = [
        nc.alloc_sbuf_tensor(f"emb_sb{ci}", [D, c1 - c0], bf16).ap()
        for ci, (c0, c1) in enumerate(dma_chunks)
    ]
    wscr = nc.alloc_sbuf_tensor("wscr", [D, D + MM], bf16).ap()
    ctxT = nc.alloc_sbuf_tensor("ctxT", [D, N], bf16).ap()
    ctx_f32 = nc.alloc_sbuf_tensor("ctx_f32", [D, N], f32).ap()
    prod = nc.alloc_sbuf_tensor("prod", [D, N * T], bf16).ap()
    if SCHEME == "wide":
        out_sb = nc.alloc_sbuf_tensor("out_sb", out_shape, bf16).ap()
    else:
        out_sb = nc.alloc_sbuf_tensor("out_sb", [D, NTILE * N], bf16).ap()

    ps_warm = nc.alloc_psum_tensor("ps_warm", [D, MM], f32).ap()
    if SCHEME == "wide":
        if QUAD:
            ps_main = [
                nc.alloc_psum_tensor(f"ps_q{j}", [D, MM], f32).ap() for j in range(3)
            ]
        else:
            ps_main = [
                nc.alloc_psum_tensor(f"ps_main{j}", [N, MM], f32).ap()
                for j in range(NPS)
            ]
    else:
        ps_tp = [
            nc.alloc_psum_tensor("ps_tp0", [D, MM], f32).ap(),
            nc.alloc_psum_tensor("ps_tp1", [D, (NTILE - 32) * N], f32).ap(),
        ]

    lda = [nc.alloc_semaphore(f"lda{g}") for g in range(2)]
    lde = [nc.alloc_semaphore(f"lde{ci}") for ci in range(n_chunks)]
    z = nc.alloc_semaphore("z")
    cxr = nc.alloc_semaphore("cxr")
    cxv = nc.alloc_semaphore("cxv")
    mm_sem = nc.alloc_semaphore("mm")
    cp = {k: nc.alloc_semaphore(f"cp_{k}") for k in CPENG}
    st = nc.alloc_semaphore("st")
    all_sems = lda + lde + [z, cxr, cxv, mm_sem] + list(cp.values()) + [st]

    # ---- entry block: all unconditional DMA issues + warmup memset ----
    for g, eng in enumerate((nc.sync, nc.scalar)):
        eng.dma_start(
            attnw_sb[:, asub[g] : asub[g + 1]], attnw[:, asub[g] : asub[g + 1]]
        ).then_inc(lda[g], 16)
    for ci, (c0, c1) in enumerate(dma_chunks):
        eng = getattr(nc, ring[ci])
        eng.dma_start(emb_sb[ci][:, :], embT[:, c0:c1]).then_inc(lde[ci], 16)
    nc.vector.memset(wscr[:, :], 0.0).then_inc(z, 1)

    def ctx_stt(h):
        """prod half h = attnw[n-half h] * w (one STT with w broadcast)."""
        in0 = _view3(attnw_sb[:, T + h * 8 * T : T + (h + 1) * 8 * T], T, 8)
        w_b = _view3(attnw_sb[:, 0:T], T, 8, bcast=True)
        pv = _view3(prod[:, h * 8 * T :], T, 8)
        return nc.vector.scalar_tensor_tensor(
            pv, in0, 1.0, w_b,
            op0=mybir.AluOpType.bypass,
            op1=mybir.AluOpType.mult,
        )

    def ctx_reduce():
        with nc.allow_low_precision("ctx reduce rounds once to bf16"):
            return nc.vector.tensor_reduce(
                ctxT[:, :], _view3(prod[:, :], T, N),
                axis=mybir.AxisListType.X, op=mybir.AluOpType.add,
            )

    # wide+shift: copy for mm-unit s lands at partition base 32*(s%4) (engine
    # writes must start on a quadrant boundary), col block 512*(s//4); stores
    # then read all partition quadrants (16 SDMA engines, half-garbage rows)
    def osb_dst(s, width):
        if OSB == "shift":
            a, b = s % 4, s // 4
            return out_sb[32 * a : 32 * a + 16, MM * b : MM * b + width]
        ci, s0, s1 = mm_list[s]
        return out_sb[:, s0 : s0 + width]

    with nc.Block() as block:

        @block.vector
        def _(v):
            v.wait_ge(lda[0], 16)
            ctx_stt(0)
            v.wait_ge(lda[1], 16)
            ctx_stt(1)
            ctx_reduce().then_inc(cxr, 1)
            if SCHEME == "wide":
                if QUAD:
                    for g in range(5):
                        hi = min(3 * (g + 1), len(mm_list))
                        v.wait_ge(mm_sem, hi)
                        nc.vector.tensor_copy(
                            out_sb[:, MM * g : MM * (g + 1)], ps_main[g % 3][:80, :]
                        ).then_inc(cp["vector"], 1)
                else:
                    for s, (ci, s0, s1) in enumerate(mm_list):
                        if CPENG[s % ne] != "vector":
                            continue
                        v.wait_ge(mm_sem, s + 1)
                        nc.vector.tensor_copy(
                            osb_dst(s, s1 - s0), ps_main[s % NPS][:, : s1 - s0]
                        ).then_inc(cp["vector"], 1)
            else:
                v.wait_ge(mm_sem, 32)
                nc.vector.tensor_copy(out_sb[:, : 32 * N], ps_tp[0][:, :]).then_inc(
                    cp["vector"], 1
                )
                v.wait_ge(mm_sem, NTILE)
                nc.vector.tensor_copy(out_sb[:, 32 * N :], ps_tp[1][:, :]).then_inc(
                    cp["vector"], 1
                )

        @block.tensor
        def _(t):
            t.wait_ge(z, 1)
            for _wi in range(N_WARMUP):
                nc.tensor.matmul(
                    ps_warm[:, :],
                    lhsT=wscr[:, :D],
                    rhs=wscr[:, D:],
                    start=True,
                    stop=True,
                )
            t.wait_ge(cxr, 1)
            if SCHEME == "wide":
                prev_ci = -1
                for s, (ci, s0, s1) in enumerate(mm_list):
                    if ci != prev_ci:
                        t.wait_ge(lde[ci], 16)
                        prev_ci = ci
                    if QUAD:
                        g, a = s // 3, s % 3
                        if a == 0 and g >= 3:
                            t.wait_ge(cp["vector"], g - 2)
                        dst = ps_main[g % 3][32 * a : 32 * a + 16, : s1 - s0]
                    else:
                        if s >= NPS:
                            q = s - NPS
                            t.wait_ge(cp[CPENG[q % ne]], q // ne + 1)
                        dst = ps_main[s % NPS][:, : s1 - s0]
                    c0 = dma_chunks[ci][0]
                    nc.tensor.matmul(
                        dst,
                        lhsT=ctxT[:, :],
                        rhs=emb_sb[ci][:, s0 - c0 : s1 - c0],
                        start=True,
                        stop=True,
                    ).then_inc(mm_sem, 1)
            else:
                prev_ci = -1
                for ti in range(NTILE):
                    ci = min(ti // 8, n_chunks - 1)
                    if ci != prev_ci:
                        t.wait_ge(lde[ci], 16)
                        prev_ci = ci
                    c0 = dma_chunks[ci][0]
                    bank, off = (0, ti) if ti < 32 else (1, ti - 32)
                    nc.tensor.matmul(
                        ps_tp[bank][:, off * N : (off + 1) * N],
                        lhsT=emb_sb[ci][:, ti * 128 - c0 : ti * 128 - c0 + 128],
                        rhs=ctxT[:, :],
                        start=True,
                        stop=True,
                    ).then_inc(mm_sem, 1)

        @block.sync
        def _(sy):
            if SCHEME == "wide" and QUAD:
                for g in (0, 2, 4):
                    sy.wait_ge(cp["vector"], g + 1)
                    nc.sync.dma_start(
                        out[:, MM * g : MM * (g + 1)],
                        out_sb[:, MM * g : MM * (g + 1)],
                    ).then_inc(st, 16)

        @block.scalar
        def _(sc):
            if SCHEME == "wide" and QUAD:
                for g in (1, 3):
                    sc.wait_ge(cp["vector"], g + 1)
                    nc.scalar.dma_start(
                        out[:, MM * g : MM * (g + 1)],
                        out_sb[:, MM * g : MM * (g + 1)],
                    ).then_inc(st, 16)
            elif SCHEME == "wide":
                acts = [s for s in range(len(mm_list)) if CPENG[s % ne] == "scalar"]
                if OSB == "shift":
                    # store1: mm-units 0-7 (col blocks 0-1); store2: 8-12
                    store_plan = [(0, 8, 0, 1024), (8, 13, 1024, 2048)]
                else:
                    store_plan = [(0, 3, 0, 1536), (3, 6, 1536, 3072),
                                  (6, 9, 3072, 4608), (9, 13, 4608, LSH)]
                done = {k: 0 for k in CPENG}
                ai = 0
                for q0, q1, b0, b1 in store_plan:
                    while ai < len(acts) and acts[ai] < q1:
                        s = acts[ai]
                        ci, s0, s1 = mm_list[s]
                        sc.wait_ge(mm_sem, s + 1)
                        nc.scalar.copy(
                            osb_dst(s, s1 - s0), ps_main[s % NPS][:, : s1 - s0]
                        ).then_inc(cp["scalar"], 1)
                        done["scalar"] += 1
                        ai += 1
                    for s in range(q0, q1):
                        k = CPENG[s % ne]
                        need = s // ne + 1
                        if k != "scalar" and need > done[k]:
                            sc.wait_ge(cp[k], need)
                            done[k] = need
                    if OSB == "shift":
                        nc.scalar.dma_start(
                            out[:, b0:b1], out_sb[:, b0:b1]
                        ).then_inc(st, 16)
                    else:
                        nc.scalar.dma_start(
                            out[:, b0:b1], out_sb[:, b0:b1]
                        ).then_inc(st, 16)
            else:
                sc.wait_ge(cp["vector"], 1)
                nc.scalar.dma_start(out[:, : 32 * N], out_sb[:, : 32 * N]).then_inc(
                    st, 16
                )
                sc.wait_ge(cp["vector"], 2)
                nc.scalar.dma_start(out[:, 32 * N :], out_sb[:, 32 * N :]).then_inc(
                    st, 16
                )
            # no completion wait: epilogue dma_reset drains the store queue

    nc.all_engine_barrier(sem_only=True)
    nc.clear_and_free_semaphores(all_sems)

    nc.compile()
    return nc


def _get_nc():
    if "nc" not in _cache:
        _cache["nc"] = _build()
    return _cache["nc"]


def _make_in_maps(self_attn, emb_table, value_w):
    import ml_dtypes

    bf = ml_dtypes.bfloat16
    self_attn = np.asarray(self_attn, dtype=np.float32)
    value_w = np.asarray(value_w, dtype=np.float32)
    # [D, T + N*T]: value_w broadcast, then d-major self_attn
    attnw = np.empty((D, AW), dtype=bf)
    attnw[:, :T] = value_w[0][None, :].astype(bf)
    attnw[:, T:] = self_attn.transpose(2, 0, 1).reshape(D, N * T).astype(bf)
    embT = np.asarray(emb_table, dtype=np.float32)[1 : L + 1].astype(bf).T  # [D, L]
    ncols = NCOLS[SCHEME]
    in_maps = []
    for k in range(NCORES):
        shard = np.zeros((D, ncols), dtype=bf)
        shard[:, :LSH] = embT[:, k * LSH : (k + 1) * LSH]
        in_maps.append({"embT": shard, "attnw": attnw})
    return in_maps


def _unshard(o):
    o = np.asarray(o)
    if SCHEME == "wide":
        if OSB == "shift":
            # quad: dram[32a+n, 512g+j] = out[n, 512*(3g+a)+j]; legacy shift
            # layout uses 4-unit groups on [128, 2048]
            na = 3 if QUAD else 4
            full = np.empty((N, LSH), dtype=np.float32)
            for s in range(13):
                a, b = s % na, s // na
                w = min(MM, LSH - s * MM)
                full[:, s * MM : s * MM + w] = o[
                    32 * a : 32 * a + 16, MM * b : MM * b + w
                ].astype(np.float32)
            return full
        return o.astype(np.float32)
    # tp: [128, 49*16] -> [49,128,16] l-major -> [LSH, N] -> [N, LSH]
    return (
        o.reshape(D, NTILE, N)
        .transpose(1, 0, 2)
        .reshape(LPAD, N)[:LSH]
        .T.astype(np.float32)
    )


def run(self_attn, emb_table, value_w, trace=False):
    from concourse.bass_utils import run_bass_kernel_spmd

    nc = _get_nc()
    in_maps = _make_in_maps(self_attn, emb_table, value_w)
    res = run_bass_kernel_spmd(nc, in_maps, list(range(NCORES)), trace=trace)
    full = np.ascontiguousarray(
        np.concatenate([_unshard(res.results[k]["out"]) for k in range(NCORES)], axis=1),
        dtype=np.float32,
    )
    return full, res


def kernel(self_attn, mat2, traj, emb_table, value_w):
    full, _ = run(self_attn, emb_table, value_w, trace=False)
    return full


# revision 17
# speedup vs baseline: 1.2717x; 1.0218x over previous
"""Trainium2 Bass kernel for nn_AttnMatching.

Reference computes:
    emb = emb_table[1:L+1]                      # [L, D]
    attn = einsum('ld,ntd->nlt', emb, self_attn)
    out  = einsum('nlt,t->nl', attn, value_w[0])

Reassociated (identical math):
    ctx[n, d] = sum_t value_w[t] * self_attn[n, t, d]    # [N, D]  (tiny)
    out[n, l] = sum_d ctx[n, d] * emb[l, d]              # [N, L]

Memory-bound: dominant traffic is streaming the embedding table.
Sharding: vocab axis L split across 8 cores (6250 cols each),
self_attn/value_w replicated, no communication.

All device traffic is bf16 (host-cast): emb 1.6 MB/core, attnw 0.44 MB,
out 0.2 MB. Matmuls run at bf16 rate (fp32 is 4 cycles/row + LOW_HIGH
double-pass). rel-err from bf16 ~3e-3, gate is 2e-2.

Per-core schedule (raw bacc, hand-rolled sems):
  - 3 DMA queues stream from the entry block: sync(HWDGE) carries the
    attnw burst (2 subs) then the last emb chunk; scalar(HWDGE) the
    first two emb chunks; gpsimd(SWDGE) the middle three.
  - ctx: per 8-n half, ONE fused multiply (scalar_tensor_tensor with a
    zero-stride broadcast AP repeating the w block) + ONE segmented
    tensor_reduce (axis=X over [128,8,100]) on DVE; ACT casts ctx_f32
    -> bf16 ctxT cross-engine (sem-gated: accumulator/pipeline drain).
  - PE: dependency-free bf16 warmups hold the clock ramp, then mains.
    Two schemes (K_SCHEME):
      wide: lhsT=ctxT [D,16] stationary, rhs=emb [D,512] -> PSUM
            [16,512] x13 over 6 banks; PSUM->SBUF copies round-robin
            DVE/ACT; out_sb repartitioned [64, 2048] (copy s writes
            partition base 16*(s%4)) so stores engage 8 SDMA engines.
      tp:   lhsT=emb tile [D,128] stationary, rhs=ctxT [D,16] moving ->
            PSUM [128,16] x49 packed into 2 banks; 2 full-width DVE
            copies; 2 full-rate [128,*] stores (host un-permutes).
  - Epilogue: sem-only all-engine barrier + semaphore clear so the NEFF
    is safe to re-execute.
"""

import os

import numpy as np

L = 50000
D = 128
T = 100
N = 16
NCORES = 8
LSH = L // NCORES          # 6250 columns per core

MM = 512                   # PSUM bank limit: fp32 out cols per matmul
SCHEME = os.environ.get("K_SCHEME", "wide")  # "wide" | "tp"
N_WARMUP = int(os.environ.get("K_N_WARMUP", "12"))
NUM_DEVICES = int(os.environ.get("K_NUM_DEVICES", str(NCORES)))
# wide-scheme out_sb layout: "shift" = [64, 2048] repartitioned (8-engine
# stores), "flat" = [16, LSH] (4-engine stores, no partition-shift copies)
OSB = os.environ.get("K_OSB", "shift")
# quad: matmul outputs land at 4 PSUM partition quadrants (tile_position via
# out AP base) so ONE 128-lane DVE copy moves 4 mm-units
QUAD = os.environ.get("K_QUAD", "1") == "1"

AW = T + N * T             # attnw cols: [w bcast | sa d-major]
NTILE = (LSH + 127) // 128          # 49 transposed tiles
LPAD = NTILE * 128                  # 6272: tp-scheme padded cols
NCOLS = {"wide": LSH, "tp": LPAD}

_cache = {}


def _chunks(total, step):
    return [(c0, min(c0 + step, total)) for c0 in range(0, total, step)]


def _view3(ap2d, ncols_inner, nrep, bcast=False):
    """[128, nrep*ncols_inner] slice -> [128, nrep, ncols_inner] view.
    bcast repeats the first ncols_inner cols nrep times (stride 0)."""
    from concourse.bass import AP

    pstep = ap2d.ap[0][0]
    step_rep = 0 if bcast else ncols_inner
    return AP(
        ap2d.tensor,
        ap2d.offset,
        [[pstep, 128], [step_rep, nrep], [1, ncols_inner]],
    )


def _build():
    import concourse.bacc as bacc
    import concourse.mybir as mybir

    f32 = mybir.dt.float32
    bf16 = mybir.dt.bfloat16

    nc = bacc.Bacc(
        "TRN2",
        target_bir_lowering=False,
        debug=False,
        enable_asserts=True,
        num_devices=NUM_DEVICES,
    )

    ncols = NCOLS[SCHEME]
    embT = nc.dram_tensor("embT", [D, ncols], bf16, kind="ExternalInput").ap()
    attnw = nc.dram_tensor("attnw", [D, AW], bf16, kind="ExternalInput").ap()
    if SCHEME == "wide":
        out_shape = ([80, 2560] if QUAD else [128, 2048]) if OSB == "shift" else [N, LSH]
    else:
        out_shape = [D, NTILE * N]
    out = nc.dram_tensor("out", out_shape, bf16, kind="ExternalOutput").ap()

    # emb chunks across the 3 DMA queues (1024-col granularity)
    bounds = [0, 1024, 2048, 3072, 4096, 5120, ncols]
    dma_chunks = list(zip(bounds[:-1], bounds[1:]))
    ring = {0: "gpsimd", 1: "gpsimd", 2: "gpsimd", 3: "sync", 4: "sync", 5: "scalar"}
    n_chunks = len(dma_chunks)

    # attnw sub-DMAs alternating queues: a0 = w + n0..3, then 4-n blocks
    asub = [0, T + 4 * T, T + 8 * T, T + 12 * T, AW]

    # wide-scheme matmul list: (chunk_idx, abs_s0, abs_s1), 512-col units
    mm_list = []
    for ci, (c0, c1) in enumerate(dma_chunks):
        for s0, s1 in _chunks(c1 - c0, MM):
            mm_list.append((ci, c0 + s0, c0 + s1))
    NPS = 6
    CPENG = ["vector", "scalar"]  # copy engine per wide mm-unit (no PSUM on gpsimd)
    ne = len(CPENG)

    attnw_sb = nc.alloc_sbuf_tensor("attnw_sb", [D, AW], bf16).ap()
    emb_sb = [
        nc.alloc_sbuf_tensor(f"emb_sb{ci}", [D, c1 - c0], bf16).ap()
        for ci, (c0, c1) in enumerate(dma_chunks)
    ]
    wscr = nc.alloc_sbuf_tensor("wscr", [D, D + MM], bf16).ap()
    ctxT = nc.alloc_sbuf_tensor("ctxT", [D, N], bf16).ap()
    ctx_f32 = nc.alloc_sbuf_tensor("ctx_f32", [D, N], f32).ap()
    prod = nc.alloc_sbuf_tensor("prod", [D, N * T], bf16).ap()
    if SCHEME == "wide":
        out_sb = nc.alloc_sbuf_tensor("out_sb", out_shape, bf16).ap()
    else:
        out_sb = nc.alloc_sbuf_tensor("out_sb", [D, NTILE * N], bf16).ap()

    ps_warm = nc.alloc_psum_tensor("ps_warm", [D, MM], f32).ap()
    if SCHEME == "wide":
        if QUAD:
            ps_main = [
                nc.alloc_psum_tensor(f"ps_q{j}", [D, MM], f32).ap() for j in range(3)
            ]
        else:
            ps_main = [
                nc.alloc_psum_tensor(f"ps_main{j}", [N, MM], f32).ap()
                for j in range(NPS)
            ]
    else:
        ps_tp = [
            nc.alloc_psum_tensor("ps_tp0", [D, MM], f32).ap(),
            nc.alloc_psum_tensor("ps_tp1", [D, (NTILE - 32) * N], f32).ap(),
        ]

    lda = [nc.alloc_semaphore(f"lda{g}") for g in range(4)]
    lde = [nc.alloc_semaphore(f"lde{ci}") for ci in range(n_chunks)]
    z = nc.alloc_semaphore("z")
    cxr = nc.alloc_semaphore("cxr")
    cxv = nc.alloc_semaphore("cxv")
    mm_sem = nc.alloc_semaphore("mm")
    cp = {k: nc.alloc_semaphore(f"cp_{k}") for k in CPENG}
    st = nc.alloc_semaphore("st")
    all_sems = lda + lde + [z, cxr, cxv, mm_sem] + list(cp.values()) + [st]

    # ---- entry block: all unconditional DMA issues + warmup memset ----
    for g, eng in enumerate((nc.sync, nc.scalar, nc.sync, nc.scalar)):
        eng.dma_start(
            attnw_sb[:, asub[g] : asub[g + 1]], attnw[:, asub[g] : asub[g + 1]]
        ).then_inc(lda[g], 16)
    for ci, (c0, c1) in enumerate(dma_chunks):
        eng = getattr(nc, ring[ci])
        eng.dma_start(emb_sb[ci][:, :], embT[:, c0:c1]).then_inc(lde[ci], 16)
    nc.vector.memset(wscr[:, :], 0.0).then_inc(z, 1)

    def ctx_stt(h):
        """prod quarter h = attnw[n-quarter h] * w (one STT with w broadcast)."""
        in0 = _view3(attnw_sb[:, T + h * 4 * T : T + (h + 1) * 4 * T], T, 4)
        w_b = _view3(attnw_sb[:, 0:T], T, 4, bcast=True)
        pv = _view3(prod[:, h * 4 * T :], T, 4)
        return nc.vector.scalar_tensor_tensor(
            pv, in0, 1.0, w_b,
            op0=mybir.AluOpType.bypass,
            op1=mybir.AluOpType.mult,
        )

    def ctx_reduce():
        with nc.allow_low_precision("ctx reduce rounds once to bf16"):
            return nc.vector.tensor_reduce(
                ctxT[:, :], _view3(prod[:, :], T, N),
                axis=mybir.AxisListType.X, op=mybir.AluOpType.add,
            )

    # wide+shift: copy for mm-unit s lands at partition base 32*(s%4) (engine
    # writes must start on a quadrant boundary), col block 512*(s//4); stores
    # then read all partition quadrants (16 SDMA engines, half-garbage rows)
    def osb_dst(s, width):
        if OSB == "shift":
            a, b = s % 4, s // 4
            return out_sb[32 * a : 32 * a + 16, MM * b : MM * b + width]
        ci, s0, s1 = mm_list[s]
        return out_sb[:, s0 : s0 + width]

    with nc.Block() as block:

        @block.vector
        def _(v):
            for h in range(4):
                v.wait_ge(lda[h], 16)
                ctx_stt(h)
            ctx_reduce().then_inc(cxr, 1)
            if SCHEME == "wide":
                if QUAD:
                    for g in (0, 2, 4):
                        hi = min(3 * (g + 1), len(mm_list))
                        v.wait_ge(mm_sem, hi)
                        dst = (out_sb[:16, MM * g : MM * g + 106]
                               if g == 4 else out_sb[:, MM * g : MM * (g + 1)])
                        srcp = (ps_main[g % 3][:16, :106]
                                if g == 4 else ps_main[g % 3][:80, :])
                        nc.vector.tensor_copy(dst, srcp).then_inc(cp["vector"], 1)
                else:
                    for s, (ci, s0, s1) in enumerate(mm_list):
                        if CPENG[s % ne] != "vector":
                            continue
                        v.wait_ge(mm_sem, s + 1)
                        nc.vector.tensor_copy(
                            osb_dst(s, s1 - s0), ps_main[s % NPS][:, : s1 - s0]
                        ).then_inc(cp["vector"], 1)
            else:
                v.wait_ge(mm_sem, 32)
                nc.vector.tensor_copy(out_sb[:, : 32 * N], ps_tp[0][:, :]).then_inc(
                    cp["vector"], 1
                )
                v.wait_ge(mm_sem, NTILE)
                nc.vector.tensor_copy(out_sb[:, 32 * N :], ps_tp[1][:, :]).then_inc(
                    cp["vector"], 1
                )

        @block.tensor
        def _(t):
            t.wait_ge(z, 1)
            for _wi in range(N_WARMUP):
                nc.tensor.matmul(
                    ps_warm[:, :],
                    lhsT=wscr[:, :D],
                    rhs=wscr[:, D:],
                    start=True,
                    stop=True,
                )
            t.wait_ge(cxr, 1)
            if SCHEME == "wide":
                prev_ci = -1
                for s, (ci, s0, s1) in enumerate(mm_list):
                    if ci != prev_ci:
                        t.wait_ge(lde[ci], 16)
                        prev_ci = ci
                    if QUAD:
                        g, a = s // 3, s % 3
                        if a == 0 and g >= 3:
                            gq = g - 3
                            eng = "vector" if gq % 2 == 0 else "scalar"
                            t.wait_ge(cp[eng], gq // 2 + 1)
                        dst = ps_main[g % 3][32 * a : 32 * a + 16, : s1 - s0]
                    else:
                        if s >= NPS:
                            q = s - NPS
                            t.wait_ge(cp[CPENG[q % ne]], q // ne + 1)
                        dst = ps_main[s % NPS][:, : s1 - s0]
                    c0 = dma_chunks[ci][0]
                    nc.tensor.matmul(
                        dst,
                        lhsT=ctxT[:, :],
                        rhs=emb_sb[ci][:, s0 - c0 : s1 - c0],
                        start=True,
                        stop=True,
                    ).then_inc(mm_sem, 1)
            else:
                prev_ci = -1
                for ti in range(NTILE):
                    ci = min(ti // 8, n_chunks - 1)
                    if ci != prev_ci:
                        t.wait_ge(lde[ci], 16)
                        prev_ci = ci
                    c0 = dma_chunks[ci][0]
                    bank, off = (0, ti) if ti < 32 else (1, ti - 32)
                    nc.tensor.matmul(
                        ps_tp[bank][:, off * N : (off + 1) * N],
                        lhsT=emb_sb[ci][:, ti * 128 - c0 : ti * 128 - c0 + 128],
                        rhs=ctxT[:, :],
                        start=True,
                        stop=True,
                    ).then_inc(mm_sem, 1)

        @block.sync
        def _(sy):
            if SCHEME == "wide" and QUAD:
                for k, g in enumerate((0, 2, 4)):
                    sy.wait_ge(cp["vector"], k + 1)
                    dst = (out[:16, MM * g : MM * g + 106]
                           if g == 4 else out[:, MM * g : MM * (g + 1)])
                    srcp = (out_sb[:16, MM * g : MM * g + 106]
                            if g == 4 else out_sb[:, MM * g : MM * (g + 1)])
                    nc.sync.dma_start(dst, srcp).then_inc(st, 16)

        @block.scalar
        def _(sc):
            if SCHEME == "wide" and QUAD:
                for g in (1, 3):
                    sc.wait_ge(mm_sem, 3 * (g + 1))
                    nc.scalar.copy(
                        out_sb[:, MM * g : MM * (g + 1)], ps_main[g % 3][:80, :]
                    ).then_inc(cp["scalar"], 1)
                    nc.scalar.dma_start(
                        out[:, MM * g : MM * (g + 1)],
                        out_sb[:, MM * g : MM * (g + 1)],
                    ).then_inc(st, 16)
            elif SCHEME == "wide":
                acts = [s for s in range(len(mm_list)) if CPENG[s % ne] == "scalar"]
                if OSB == "shift":
                    # store1: mm-units 0-7 (col blocks 0-1); store2: 8-12
                    store_plan = [(0, 8, 0, 1024), (8, 13, 1024, 2048)]
                else:
                    store_plan = [(0, 3, 0, 1536), (3, 6, 1536, 3072),
                                  (6, 9, 3072, 4608), (9, 13, 4608, LSH)]
                done = {k: 0 for k in CPENG}
                ai = 0
                for q0, q1, b0, b1 in store_plan:
                    while ai < len(acts) and acts[ai] < q1:
                        s = acts[ai]
                        ci, s0, s1 = mm_list[s]
                        sc.wait_ge(mm_sem, s + 1)
                        nc.scalar.copy(
                            osb_dst(s, s1 - s0), ps_main[s % NPS][:, : s1 - s0]
                        ).then_inc(cp["scalar"], 1)
                        done["scalar"] += 1
                        ai += 1
                    for s in range(q0, q1):
                        k = CPENG[s % ne]
                        need = s // ne + 1
                        if k != "scalar" and need > done[k]:
                            sc.wait_ge(cp[k], need)
                            done[k] = need
                    if OSB == "shift":
                        nc.scalar.dma_start(
                            out[:, b0:b1], out_sb[:, b0:b1]
                        ).then_inc(st, 16)
                    else:
                        nc.scalar.dma_start(
                            out[:, b0:b1], out_sb[:, b0:b1]
                        ).then_inc(st, 16)
            else:
                sc.wait_ge(cp["vector"], 1)
                nc.scalar.dma_start(out[:, : 32 * N], out_sb[:, : 32 * N]).then_inc(
                    st, 16
                )
                sc.wait_ge(cp["vector"], 2)
                nc.scalar.dma_start(out[:, 32 * N :], out_sb[:, 32 * N :]).then_inc(
                    st, 16
                )
            # no completion wait: epilogue dma_reset drains the store queue

    nc.all_engine_barrier(sem_only=True)
    nc.clear_and_free_semaphores(all_sems)

    nc.compile()
    return nc


def _get_nc():
    if "nc" not in _cache:
        _cache["nc"] = _build()
    return _cache["nc"]


def _make_in_maps(self_attn, emb_table, value_w):
    import ml_dtypes

    bf = ml_dtypes.bfloat16
    self_attn = np.asarray(self_attn, dtype=np.float32)
    value_w = np.asarray(value_w, dtype=np.float32)
    # [D, T + N*T]: value_w broadcast, then d-major self_attn
    attnw = np.empty((D, AW), dtype=bf)
    attnw[:, :T] = value_w[0][None, :].astype(bf)
    attnw[:, T:] = self_attn.transpose(2, 0, 1).reshape(D, N * T).astype(bf)
    embT = np.asarray(emb_table, dtype=np.float32)[1 : L + 1].astype(bf).T  # [D, L]
    ncols = NCOLS[SCHEME]
    in_maps = []
    for k in range(NCORES):
        shard = np.zeros((D, ncols), dtype=bf)
        shard[:, :LSH] = embT[:, k * LSH : (k + 1) * LSH]
        in_maps.append({"embT": shard, "attnw": attnw})
    return in_maps


def _unshard(o):
    o = np.asarray(o)
    if SCHEME == "wide":
        if OSB == "shift":
            # quad: dram[32a+n, 512g+j] = out[n, 512*(3g+a)+j]; legacy shift
            # layout uses 4-unit groups on [128, 2048]
            na = 3 if QUAD else 4
            full = np.empty((N, LSH), dtype=np.float32)
            for s in range(13):
                a, b = s % na, s // na
                w = min(MM, LSH - s * MM)
                full[:, s * MM : s * MM + w] = o[
                    32 * a : 32 * a + 16, MM * b : MM * b + w
                ].astype(np.float32)
            return full
        return o.astype(np.float32)
    # tp: [128, 49*16] -> [49,128,16] l-major -> [LSH, N] -> [N, LSH]
    return (
        o.reshape(D, NTILE, N)
        .transpose(1, 0, 2)
        .reshape(LPAD, N)[:LSH]
        .T.astype(np.float32)
    )


def run(self_attn, emb_table, value_w, trace=False):
    from concourse.bass_utils import run_bass_kernel_spmd

    nc = _get_nc()
    in_maps = _make_in_maps(self_attn, emb_table, value_w)
    res = run_bass_kernel_spmd(nc, in_maps, list(range(NCORES)), trace=trace)
    full = np.ascontiguousarray(
        np.concatenate([_unshard(res.results[k]["out"]) for k in range(NCORES)], axis=1),
        dtype=np.float32,
    )
    return full, res


def kernel(self_attn, mat2, traj, emb_table, value_w):
    full, _ = run(self_attn, emb_table, value_w, trace=False)
    return full
